# revision 6
# baseline (speedup 1.0000x reference)
"""GCNII message-passing kernel for 8 Trainium2 NeuronCores.

Strategy (dest-sharded SpMM, indicator-matmul segment sum, dma_gather):
  - Nodes sharded across 8 cores (12500 each, padded to 12544 = 98*128).
  - Edges partitioned by destination core. Within a core, edges are
    grouped by (source-quarter q, 128-dest block b) and padded into
    128-edge tiles; tiles are laid out quarter-major so each dma_gather
    batch reads one 25088-row sub-table with int16 indices.
  - Self-loop term folded in as explicit self-edges.
  - g = dinv * h is the gathered quantity, so the edge weight
    norm_e = dinv[dst]*dinv[src] factors as (0.9*dinv[dst]) -> folded
    into the indicator matrix S[e,d] = (iota==dst_local)*0.9*dinv[dst]
    (one DVE tensor_scalar per tile).
  - Per tile one PE matmul accumulates sumT[h,d] += G[e,h]^T S[e,d] in
    PSUM; per (q,b) the PSUM partial is added into an SBUF accumulator.
  - Block epilogue: out[d,:] = accT^T... = lhsT(accT)@W' + lhsT(h0T)@(0.1W')
    via two PSUM-accumulated matmuls (W' = theta*W + (1-theta)*I), then
    g_next = dinv * relu(out) on the scalar engine.
  - g is all-gathered across cores between layers (ping-pong buffers).
"""

import math

import numpy as np

import concourse.bass as bass
import concourse.bacc as bacc
import concourse.mybir as mybir
import concourse.tile as tile
from concourse import library_config
from concourse.bass_utils import run_bass_kernel_spmd

N = 100000
FIN = 256
H = 128
L = 4
NCORES = 8
SHARD = N // NCORES            # 12500
NBLK = (SHARD + 127) // 128    # 98
SPAD = NBLK * 128              # 12544
NGF = NCORES * SPAD            # 100352 rows in gathered g table
NQ = 4
QROW = NGF // NQ               # 25088 rows per int16-addressable sub-table
LAMDA = 0.5
ALPHA = 0.1
import os
KB = int(os.environ.get("GCN_KB", "8"))   # edge tiles per batched dma_gather

F32 = mybir.dt.float32
I16 = mybir.dt.int16

LAST_RESULT = None


def _pg(v):
    """global node id -> padded row id in the all-gathered g table"""
    return SPAD * (v // SHARD) + (v % SHARD)


def _preprocess(edge_index):
    ei = np.asarray(edge_index).astype(np.int64)
    row, col = ei[0], ei[1]

    deg = (np.bincount(row, minlength=N) + 1.0).astype(np.float32)
    dinv = (1.0 / np.sqrt(deg.astype(np.float64))).astype(np.float32)

    core_of = row // SHARD
    selfd = np.arange(SHARD, dtype=np.int64)

    edata = []
    counts = np.zeros((NCORES, NQ * NBLK), dtype=np.int64)
    for k in range(NCORES):
        m = core_of == k
        dl = row[m] - k * SHARD
        src = col[m]
        a_dl = np.concatenate([dl, selfd])
        a_srcpg = _pg(np.concatenate([src, k * SHARD + selfd]))
        a_key = (a_srcpg // QROW) * NBLK + (a_dl >> 7)
        counts[k] = np.bincount(a_key, minlength=NQ * NBLK)
        edata.append((a_dl, a_srcpg, a_key))

    nt = (counts.max(axis=0) + 127) // 128          # [NQ*NBLK] tiles per group
    off = np.concatenate([[0], np.cumsum(nt)])      # tile offset per group
    T = int(off[-1])

    per_core = []
    for k in range(NCORES):
        a_dl, a_srcpg, a_key = edata[k]
        order = np.argsort(a_key, kind="stable")
        a_dl, a_srcpg, a_key = a_dl[order], a_srcpg[order], a_key[order]
        starts = np.concatenate([[0], np.cumsum(counts[k])])
        rank = np.arange(len(a_key)) - starts[a_key]
        slot = off[a_key] * 128 + rank

        colx = np.zeros(T * 128, dtype=np.int16)
        dstf = np.full(T * 128, -1.0, dtype=np.float32)
        s2f = np.zeros(T * 128, dtype=np.float32)
        colx[slot] = (a_srcpg % QROW).astype(np.int16)
        dstf[slot] = (a_dl & 127).astype(np.float32)
        s2f[slot] = (1.0 - ALPHA) * dinv[k * SHARD + a_dl]

        # idx wrap: logical i at [i%16, i//16], replicated to 128 partitions
        colx16 = np.tile(colx.reshape(T * 8, 16).T, (8, 1))   # [128, 8T]
        dstf = np.ascontiguousarray(dstf.reshape(T, 128).T)   # [128, T]
        s2f = np.ascontiguousarray(s2f.reshape(T, 128).T)

        dv = np.zeros(SPAD, dtype=np.float32)
        dv[:SHARD] = dinv[k * SHARD:(k + 1) * SHARD]
        dvb = np.ascontiguousarray(dv.reshape(NBLK, 128).T)   # [128, NBLK]

        per_core.append(dict(colx16=np.ascontiguousarray(colx16),
                             dstf=dstf, s2f=s2f, dvb=dvb))

    return per_core, T, nt, off


def _build(T, nt, off):
    nc = bacc.Bacc("TRN2", target_bir_lowering=False, debug=False,
                   num_devices=NCORES)

    xT_t = nc.dram_tensor("xT", [FIN, SPAD], F32, kind="ExternalInput")
    fcw_t = nc.dram_tensor("fcw", [FIN, H], F32, kind="ExternalInput")
    fcb_t = nc.dram_tensor("fcb", [H, 1], F32, kind="ExternalInput")
    wp_t = nc.dram_tensor("wp", [L * H, H], F32, kind="ExternalInput")
    w01_t = nc.dram_tensor("w01", [L * H, H], F32, kind="ExternalInput")
    colx_t = nc.dram_tensor("colx", [128, 8 * T], I16, kind="ExternalInput")
    dstf_t = nc.dram_tensor("dstf", [128, T], F32, kind="ExternalInput")
    s2f_t = nc.dram_tensor("s2f", [128, T], F32, kind="ExternalInput")
    dvb_t = nc.dram_tensor("dvb", [128, NBLK], F32, kind="ExternalInput")
    iota_t = nc.dram_tensor("iotaf", [128, 128], F32, kind="ExternalInput")
    ident_t = nc.dram_tensor("ident", [128, 128], F32, kind="ExternalInput")
    out_t = nc.dram_tensor("out", [SPAD, H], F32, kind="ExternalOutput")

    ag_in = nc.dram_tensor("ag_in", [SPAD, H], F32)
    g_full = [nc.dram_tensor(f"g_full{i}", [NGF, H], F32, addr_space="Shared")
              for i in range(2)]

    groups = [list(range(NCORES))]

    # first quarter with any tiles, per block (same across cores)
    first_q = [min(q for q in range(NQ) if nt[q * NBLK + b] > 0)
               for b in range(NBLK)]

    with tile.TileContext(nc) as tc:
        with (
            tc.tile_pool(name="const", bufs=1) as cpool,
            tc.tile_pool(name="meta", bufs=1) as mpool,
            tc.tile_pool(name="h0", bufs=1) as hpool,
            tc.tile_pool(name="acc", bufs=1) as apool,
            tc.tile_pool(name="xc", bufs=3) as xpool,
            tc.tile_pool(name="ix", bufs=3) as ipool,
            tc.tile_pool(name="gb", bufs=2) as gpool,
            tc.tile_pool(name="s", bufs=6) as spool,
            tc.tile_pool(name="eout", bufs=4) as epool,
            tc.tile_pool(name="pT", bufs=4, space="PSUM") as pT,
            tc.tile_pool(name="p2", bufs=2, space="PSUM") as p2,
        ):
            nc.gpsimd.load_library(library_config.mlp)

            iota = cpool.tile([128, 128], F32)
            nc.sync.dma_start(out=iota[:], in_=iota_t[:])
            ident = cpool.tile([128, 128], F32)
            nc.sync.dma_start(out=ident[:], in_=ident_t[:])
            fcw0 = cpool.tile([128, H], F32)
            nc.sync.dma_start(out=fcw0[:], in_=fcw_t[0:128, :])
            fcw1 = cpool.tile([128, H], F32)
            nc.sync.dma_start(out=fcw1[:], in_=fcw_t[128:256, :])
            fcb = cpool.tile([128, 1], F32)
            nc.sync.dma_start(out=fcb[:], in_=fcb_t[:])
            dvb = cpool.tile([128, NBLK], F32)
            nc.sync.dma_start(out=dvb[:], in_=dvb_t[:])
            wp = cpool.tile([128, L * H], F32)
            w01 = cpool.tile([128, L * H], F32)
            for l in range(L):
                nc.sync.dma_start(out=wp[:, l * H:(l + 1) * H],
                                  in_=wp_t[l * H:(l + 1) * H, :])
                nc.sync.dma_start(out=w01[:, l * H:(l + 1) * H],
                                  in_=w01_t[l * H:(l + 1) * H, :])
            dstf = mpool.tile([128, T], F32)
            nc.sync.dma_start(out=dstf[:], in_=dstf_t[:])
            s2f = mpool.tile([128, T], F32)
            nc.sync.dma_start(out=s2f[:], in_=s2f_t[:])

            h0T = hpool.tile([128, SPAD], F32)
            accT = apool.tile([128, SPAD], F32)

            # ---- phase A: h0T = fcw^T @ xT + b ; g0 = dinv*h0 ---------
            for b in range(NBLK):
                cs = slice(b * 128, (b + 1) * 128)
                xc0 = xpool.tile([128, 128], F32, tag="xc0")
                nc.sync.dma_start(out=xc0[:], in_=xT_t[0:128, cs])
                xc1 = xpool.tile([128, 128], F32, tag="xc1")
                nc.sync.dma_start(out=xc1[:], in_=xT_t[128:256, cs])
                ph = pT.tile([128, 128], F32, tag="psT")
                nc.tensor.matmul(out=ph[:], lhsT=fcw0[:], rhs=xc0[:],
                                 start=True, stop=False)
                nc.tensor.matmul(out=ph[:], lhsT=fcw1[:], rhs=xc1[:],
                                 start=False, stop=True)
                nc.vector.tensor_scalar(out=h0T[:, cs], in0=ph[:],
                                        scalar1=fcb[:, :1], scalar2=None,
                                        op0=mybir.AluOpType.add)
                pg0 = pT.tile([128, 128], F32, tag="psT")
                nc.tensor.transpose(out=pg0[:], in_=h0T[:, cs], identity=ident[:])
                g0 = epool.tile([128, H], F32, tag="gnew")
                nc.scalar.activation(out=g0[:], in_=pg0[:],
                                     func=mybir.ActivationFunctionType.Copy,
                                     scale=dvb[:, b:b + 1])
                nc.sync.dma_start(out=ag_in[b * 128:(b + 1) * 128, :], in_=g0[:])

            nc.gpsimd.collective_compute(
                "AllGather", mybir.AluOpType.bypass, replica_groups=groups,
                ins=[ag_in[:]], outs=[g_full[0][:]])

            # ---- phase B: 4 GCNII layers ------------------------------
            for l in range(L):
                gsrc = g_full[l % 2]
                gbufs = {}
                for q in range(NQ):
                    qt0, qt1 = int(off[q * NBLK]), int(off[(q + 1) * NBLK])
                    gq = gsrc[q * QROW:(q + 1) * QROW, :]
                    for t0 in range(qt0, qt1, KB):
                        t1 = min(qt1, t0 + KB)
                        nb = t1 - t0
                        ix = ipool.tile([128, KB * 8], I16, tag="ix")
                        nc.sync.dma_start(out=ix[:, :nb * 8],
                                          in_=colx_t[:, t0 * 8:t1 * 8])
                        gb = gpool.tile([128, KB * 128], F32, tag="gb")
                        nc.gpsimd.dma_gather(
                            gb[:, :nb * 128].rearrange("p (a b) -> p a b", b=128),
                            gq, ix[:, :nb * 8], nb * 128, nb * 128, 128)
                        gbufs[t0] = gb

                    for b in range(NBLK):
                        g_i = q * NBLK + b
                        tb0, tb1 = int(off[g_i]), int(off[g_i + 1])
                        if tb1 == tb0:
                            continue
                        psT = pT.tile([128, 128], F32, tag="psT")
                        for t in range(tb0, tb1):
                            bstart = qt0 + ((t - qt0) // KB) * KB
                            gb = gbufs[bstart]
                            j = t - bstart
                            s_t = spool.tile([128, 128], F32, tag="s")
                            nc.vector.tensor_scalar(
                                out=s_t[:], in0=iota[:],
                                scalar1=dstf[:, t:t + 1],
                                scalar2=s2f[:, t:t + 1],
                                op0=mybir.AluOpType.is_equal,
                                op1=mybir.AluOpType.mult)
                            nc.tensor.matmul(
                                out=psT[:], lhsT=gb[:, j * 128:(j + 1) * 128],
                                rhs=s_t[:],
                                start=(t == tb0), stop=(t == tb1 - 1))
                        bs = slice(b * 128, (b + 1) * 128)
                        if q == first_q[b]:
                            nc.vector.tensor_copy(out=accT[:, bs], in_=psT[:])
                        else:
                            nc.vector.tensor_tensor(
                                out=accT[:, bs], in0=accT[:, bs], in1=psT[:],
                                op=mybir.AluOpType.add)

                ls = slice(l * H, (l + 1) * H)
                for b in range(NBLK):
                    bs = slice(b * 128, (b + 1) * 128)
                    ps2 = p2.tile([128, 128], F32, tag="ps2")
                    nc.tensor.matmul(out=ps2[:], lhsT=accT[:, bs], rhs=wp[:, ls],
                                     start=True, stop=False)
                    nc.tensor.matmul(out=ps2[:], lhsT=h0T[:, bs], rhs=w01[:, ls],
                                     start=False, stop=True)
                    if l < L - 1:
                        gn = epool.tile([128, H], F32, tag="gnew")
                        nc.scalar.activation(
                            out=gn[:], in_=ps2[:],
                            func=mybir.ActivationFunctionType.Relu,
                            scale=dvb[:, b:b + 1])
                        nc.sync.dma_start(out=ag_in[bs, :], in_=gn[:])
                    else:
                        ob = epool.tile([128, H], F32, tag="gnew")
                        nc.scalar.copy(out=ob[:], in_=ps2[:])
                        nc.sync.dma_start(out=out_t[bs, :], in_=ob[:])
                if l < L - 1:
                    nc.gpsimd.collective_compute(
                        "AllGather", mybir.AluOpType.bypass,
                        replica_groups=groups,
                        ins=[ag_in[:]], outs=[g_full[(l + 1) % 2][:]])

    nc.compile()
    return nc


def kernel(x, fc_w, fc_b, Ws, edge_index):
    global LAST_RESULT
    x = np.asarray(x, dtype=np.float32)
    fc_w = np.asarray(fc_w, dtype=np.float32)
    fc_b = np.asarray(fc_b, dtype=np.float32)
    Ws = np.asarray(Ws, dtype=np.float32)

    per_core, T, nt, off = _preprocess(edge_index)

    eye = np.eye(H, dtype=np.float32)
    wp = np.empty((L * H, H), dtype=np.float32)
    for l in range(1, L + 1):
        theta = math.log(LAMDA / l + 1.0)
        wp[(l - 1) * H:l * H] = theta * Ws[l - 1] + (1.0 - theta) * eye
    w01 = (ALPHA * wp).astype(np.float32)

    iota = np.broadcast_to(np.arange(128, dtype=np.float32), (128, 128)).copy()
    ident = np.eye(128, dtype=np.float32)
    fcb2 = fc_b.reshape(H, 1).copy()

    nc = _build(T, nt, off)

    in_maps = []
    for k in range(NCORES):
        xT = np.zeros((FIN, SPAD), dtype=np.float32)
        xT[:, :SHARD] = x[k * SHARD:(k + 1) * SHARD].T
        pc = per_core[k]
        in_maps.append(dict(
            xT=xT, fcw=fc_w, fcb=fcb2, wp=wp, w01=w01,
            colx=pc["colx16"], dstf=pc["dstf"], s2f=pc["s2f"], dvb=pc["dvb"],
            iotaf=iota, ident=ident,
        ))

    res = run_bass_kernel_spmd(nc, in_maps, core_ids=list(range(NCORES)))
    LAST_RESULT = res

    out = np.concatenate(
        [res.results[k]["out"][:SHARD] for k in range(NCORES)], axis=0)
    return out


# revision 12
# speedup vs baseline: 1.5436x; 1.5436x over previous
"""GCNII message-passing kernel for 8 Trainium2 NeuronCores.

Strategy (dest-sharded SpMM, indicator-matmul segment sum, dma_gather):
  - Nodes sharded across 8 cores (12500 each, padded to 12544 = 98*128).
  - Edges partitioned by destination core. Within a core, edges are
    grouped by (source-quarter q, 128-dest block b) and padded into
    128-edge tiles; tiles are laid out quarter-major so each dma_gather
    batch reads one 25088-row sub-table with int16 indices.
  - Self-loop term folded in as explicit self-edges.
  - g = dinv * h is the gathered quantity, so the edge weight
    norm_e = dinv[dst]*dinv[src] factors as (0.9*dinv[dst]) -> folded
    into the indicator matrix S[e,d] = (iota==dst_local)*0.9*dinv[dst]
    (one DVE tensor_scalar per tile).
  - Per tile one PE matmul accumulates sumT[h,d] += G[e,h]^T S[e,d] in
    PSUM; per (q,b) the PSUM partial is added into an SBUF accumulator.
  - Block epilogue: out[d,:] = accT^T... = lhsT(accT)@W' + lhsT(h0T)@(0.1W')
    via two PSUM-accumulated matmuls (W' = theta*W + (1-theta)*I), then
    g_next = dinv * relu(out) on the scalar engine.
  - g is all-gathered across cores between layers (ping-pong buffers).
"""

import math

import numpy as np

import concourse.bass as bass
import concourse.bacc as bacc
import concourse.mybir as mybir
import concourse.tile as tile
from concourse import library_config
from concourse.bass_utils import run_bass_kernel_spmd

N = 100000
FIN = 256
H = 128
L = 4
NCORES = 8
SHARD = N // NCORES            # 12500
NBLK = (SHARD + 127) // 128    # 98
SPAD = NBLK * 128              # 12544
NGF = NCORES * SPAD            # 100352 rows in gathered g table
NQ = 4
QROW = NGF // NQ               # 25088 rows per int16-addressable sub-table
LAMDA = 0.5
ALPHA = 0.1
import os
KB = int(os.environ.get("GCN_KB", "16"))  # edge tiles per batched dma_gather
NL = int(os.environ.get("GCN_L", "4"))    # layers to actually run (bisect)
NOCC = os.environ.get("GCN_NOCC", "0") == "1"     # skip collectives (bisect)
NOGATHER = os.environ.get("GCN_NOGATHER", "0") == "1"  # plain DMA instead of gather
NOS = os.environ.get("GCN_NOS", "0") == "1"       # skip S-build (bisect)
NOMM = os.environ.get("GCN_NOMM", "0") == "1"     # skip edge matmuls (bisect)
SP = os.environ.get("GCN_SP", "0") == "1"         # dma_gather single_packet
WS = int(os.environ.get("GCN_WS", "8"))           # wide S-build chunk (0=per-tile)

F32 = mybir.dt.float32
I16 = mybir.dt.int16

LAST_RESULT = None
LAST_NC = None
LAST_IN_MAPS = None


def _pg(v):
    """global node id -> padded row id in the all-gathered g table"""
    return SPAD * (v // SHARD) + (v % SHARD)


def _preprocess(edge_index):
    ei = np.asarray(edge_index).astype(np.int64)
    row, col = ei[0], ei[1]

    deg = (np.bincount(row, minlength=N) + 1.0).astype(np.float32)
    dinv = (1.0 / np.sqrt(deg.astype(np.float64))).astype(np.float32)

    core_of = row // SHARD
    selfd = np.arange(SHARD, dtype=np.int64)

    edata = []
    counts = np.zeros((NCORES, NQ * NBLK), dtype=np.int64)
    for k in range(NCORES):
        m = core_of == k
        dl = row[m] - k * SHARD
        src = col[m]
        a_dl = np.concatenate([dl, selfd])
        a_srcpg = _pg(np.concatenate([src, k * SHARD + selfd]))
        a_key = (a_srcpg // QROW) * NBLK + (a_dl >> 7)
        counts[k] = np.bincount(a_key, minlength=NQ * NBLK)
        edata.append((a_dl, a_srcpg, a_key))

    nt = (counts.max(axis=0) + 127) // 128          # [NQ*NBLK] tiles per group
    off = np.concatenate([[0], np.cumsum(nt)])      # tile offset per group
    T = int(off[-1])

    per_core = []
    for k in range(NCORES):
        a_dl, a_srcpg, a_key = edata[k]
        order = np.argsort(a_key, kind="stable")
        a_dl, a_srcpg, a_key = a_dl[order], a_srcpg[order], a_key[order]
        starts = np.concatenate([[0], np.cumsum(counts[k])])
        rank = np.arange(len(a_key)) - starts[a_key]
        slot = off[a_key] * 128 + rank

        colx = np.zeros(T * 128, dtype=np.int16)
        dstf = np.full(T * 128, -1.0, dtype=np.float32)
        s2f = np.zeros(T * 128, dtype=np.float32)
        colx[slot] = (a_srcpg % QROW).astype(np.int16)
        dstf[slot] = (a_dl & 127).astype(np.float32)
        s2f[slot] = (1.0 - ALPHA) * dinv[k * SHARD + a_dl]

        # idx wrap: logical i at [i%16, i//16], replicated to 128 partitions
        colx16 = np.tile(colx.reshape(T * 8, 16).T, (8, 1))   # [128, 8T]
        dstf = np.ascontiguousarray(dstf.reshape(T, 128).T)   # [128, T]
        s2f = np.ascontiguousarray(s2f.reshape(T, 128).T)

        dv = np.zeros(SPAD, dtype=np.float32)
        dv[:SHARD] = dinv[k * SHARD:(k + 1) * SHARD]
        dvb = np.ascontiguousarray(dv.reshape(NBLK, 128).T)   # [128, NBLK]

        per_core.append(dict(colx16=np.ascontiguousarray(colx16),
                             dstf=dstf, s2f=s2f, dvb=dvb))

    return per_core, T, nt, off


def _build(T, nt, off):
    nc = bacc.Bacc("TRN2", target_bir_lowering=False, debug=False,
                   num_devices=NCORES)

    xT_t = nc.dram_tensor("xT", [FIN, SPAD], F32, kind="ExternalInput")
    fcw_t = nc.dram_tensor("fcw", [FIN, H], F32, kind="ExternalInput")
    fcb_t = nc.dram_tensor("fcb", [H, 1], F32, kind="ExternalInput")
    wp_t = nc.dram_tensor("wp", [L * H, H], F32, kind="ExternalInput")
    w01_t = nc.dram_tensor("w01", [L * H, H], F32, kind="ExternalInput")
    colx_t = nc.dram_tensor("colx", [128, 8 * T], I16, kind="ExternalInput")
    dstf_t = nc.dram_tensor("dstf", [128, T], F32, kind="ExternalInput")
    s2f_t = nc.dram_tensor("s2f", [128, T], F32, kind="ExternalInput")
    dvb_t = nc.dram_tensor("dvb", [128, NBLK], F32, kind="ExternalInput")
    iota_t = nc.dram_tensor("iotaf", [128, 128], F32, kind="ExternalInput")
    ident_t = nc.dram_tensor("ident", [128, 128], F32, kind="ExternalInput")
    out_t = nc.dram_tensor("out", [SPAD, H], F32, kind="ExternalOutput")

    ag_in = nc.dram_tensor("ag_in", [SPAD, H], F32)
    g_full = [nc.dram_tensor(f"g_full{i}", [NGF, H], F32, addr_space="Shared")
              for i in range(2)]

    groups = [list(range(NCORES))]

    # first quarter with any tiles, per block (same across cores)
    first_q = [min(q for q in range(NQ) if nt[q * NBLK + b] > 0)
               for b in range(NBLK)]

    with tile.TileContext(nc) as tc:
        with (
            tc.tile_pool(name="const", bufs=1) as cpool,
            tc.tile_pool(name="meta", bufs=1) as mpool,
            tc.tile_pool(name="h0", bufs=1) as hpool,
            tc.tile_pool(name="acc", bufs=1) as apool,
            tc.tile_pool(name="xc", bufs=3) as xpool,
            tc.tile_pool(name="ix", bufs=3) as ipool,
            tc.tile_pool(name="gb", bufs=2) as gpool,
            tc.tile_pool(name="s", bufs=6) as spool,
            tc.tile_pool(name="eout", bufs=4) as epool,
            tc.tile_pool(name="pT", bufs=4, space="PSUM") as pT,
            tc.tile_pool(name="p2", bufs=2, space="PSUM") as p2,
        ):
            nc.gpsimd.load_library(library_config.mlp)

            iota = cpool.tile([128, 128], F32)
            nc.sync.dma_start(out=iota[:], in_=iota_t[:])
            ident = cpool.tile([128, 128], F32)
            nc.sync.dma_start(out=ident[:], in_=ident_t[:])
            fcw0 = cpool.tile([128, H], F32)
            nc.sync.dma_start(out=fcw0[:], in_=fcw_t[0:128, :])
            fcw1 = cpool.tile([128, H], F32)
            nc.sync.dma_start(out=fcw1[:], in_=fcw_t[128:256, :])
            fcb = cpool.tile([128, 1], F32)
            nc.sync.dma_start(out=fcb[:], in_=fcb_t[:])
            dvb = cpool.tile([128, NBLK], F32)
            nc.sync.dma_start(out=dvb[:], in_=dvb_t[:])
            wp = cpool.tile([128, L * H], F32)
            w01 = cpool.tile([128, L * H], F32)
            for l in range(L):
                nc.sync.dma_start(out=wp[:, l * H:(l + 1) * H],
                                  in_=wp_t[l * H:(l + 1) * H, :])
                nc.sync.dma_start(out=w01[:, l * H:(l + 1) * H],
                                  in_=w01_t[l * H:(l + 1) * H, :])
            dstf = mpool.tile([128, T], F32)
            nc.sync.dma_start(out=dstf[:], in_=dstf_t[:])
            s2f = mpool.tile([128, T], F32)
            nc.sync.dma_start(out=s2f[:], in_=s2f_t[:])

            h0T = hpool.tile([128, SPAD], F32)
            accT = apool.tile([128, SPAD], F32)

            # ---- phase A: h0T = fcw^T @ xT + b ; g0 = dinv*h0 ---------
            for b in range(NBLK):
                cs = slice(b * 128, (b + 1) * 128)
                xc0 = xpool.tile([128, 128], F32, tag="xc0")
                nc.sync.dma_start(out=xc0[:], in_=xT_t[0:128, cs])
                xc1 = xpool.tile([128, 128], F32, tag="xc1")
                nc.sync.dma_start(out=xc1[:], in_=xT_t[128:256, cs])
                ph = pT.tile([128, 128], F32, tag="psT")
                nc.tensor.matmul(out=ph[:], lhsT=fcw0[:], rhs=xc0[:],
                                 start=True, stop=False)
                nc.tensor.matmul(out=ph[:], lhsT=fcw1[:], rhs=xc1[:],
                                 start=False, stop=True)
                nc.vector.tensor_scalar(out=h0T[:, cs], in0=ph[:],
                                        scalar1=fcb[:, :1], scalar2=None,
                                        op0=mybir.AluOpType.add)
                pg0 = pT.tile([128, 128], F32, tag="psT")
                nc.tensor.transpose(out=pg0[:], in_=h0T[:, cs], identity=ident[:])
                g0 = epool.tile([128, H], F32, tag="gnew")
                nc.scalar.activation(out=g0[:], in_=pg0[:],
                                     func=mybir.ActivationFunctionType.Copy,
                                     scale=dvb[:, b:b + 1])
                nc.sync.dma_start(out=ag_in[b * 128:(b + 1) * 128, :], in_=g0[:])

            if not NOCC:
                nc.gpsimd.collective_compute(
                    "AllGather", mybir.AluOpType.bypass, replica_groups=groups,
                    ins=[ag_in[:]], outs=[g_full[0][:]])

            # ---- phase B: 4 GCNII layers ------------------------------
            for l in range(NL):
                gsrc = g_full[l % 2]
                gbufs = {}
                for q in range(NQ):
                    qt0, qt1 = int(off[q * NBLK]), int(off[(q + 1) * NBLK])
                    gq = gsrc[q * QROW:(q + 1) * QROW, :]
                    for t0 in range(qt0, qt1, KB):
                        t1 = min(qt1, t0 + KB)
                        nb = t1 - t0
                        ix = ipool.tile([128, KB * 8], I16, tag="ix")
                        nc.sync.dma_start(out=ix[:, :nb * 8],
                                          in_=colx_t[:, t0 * 8:t1 * 8])
                        gb = gpool.tile([128, KB * 128], F32, tag="gb")
                        if NOGATHER:
                            nc.sync.dma_start(
                                out=gb[:, :nb * 128].rearrange("p (a b) -> p a b", b=128),
                                in_=gq[0:nb * 128, :].rearrange("(a p) b -> p a b", p=128))
                        else:
                            nc.gpsimd.dma_gather(
                                gb[:, :nb * 128].rearrange("p (a b) -> p a b", b=128),
                                gq, ix[:, :nb * 8], nb * 128, nb * 128, 128,
                                single_packet=SP)
                        gbufs[t0] = gb

                    for b in range(NBLK):
                        g_i = q * NBLK + b
                        tb0, tb1 = int(off[g_i]), int(off[g_i + 1])
                        if tb1 == tb0:
                            continue
                        psT = pT.tile([128, 128], F32, tag="psT")
                        swide = {}
                        if WS and not NOS:
                            for w0 in range(tb0, tb1, WS):
                                w1 = min(tb1, w0 + WS)
                                nw = w1 - w0
                                sw = spool.tile([128, WS * 128], F32, tag="sw")
                                sw3 = sw[:, :nw * 128].rearrange(
                                    "p (a f) -> p a f", f=128)
                                nc.vector.tensor_tensor(
                                    out=sw3,
                                    in0=iota[:].rearrange("p (a f) -> p a f", a=1)
                                        .to_broadcast([128, nw, 128]),
                                    in1=dstf[:, w0:w1].rearrange(
                                        "p (a f) -> p a f", f=1)
                                        .to_broadcast([128, nw, 128]),
                                    op=mybir.AluOpType.is_equal)
                                nc.vector.tensor_tensor(
                                    out=sw3, in0=sw3,
                                    in1=s2f[:, w0:w1].rearrange(
                                        "p (a f) -> p a f", f=1)
                                        .to_broadcast([128, nw, 128]),
                                    op=mybir.AluOpType.mult)
                                swide[w0] = sw
                        for t in range(tb0, tb1):
                            bstart = qt0 + ((t - qt0) // KB) * KB
                            gb = gbufs[bstart]
                            j = t - bstart
                            if WS and not NOS:
                                w0 = tb0 + ((t - tb0) // WS) * WS
                                sw = swide[w0]
                                s_ap = sw[:, (t - w0) * 128:(t - w0 + 1) * 128]
                            elif NOS:
                                s_t = spool.tile([128, 128], F32, tag="s")
                                if t == tb0:
                                    nc.vector.tensor_copy(out=s_t[:], in_=iota[:])
                                s_ap = iota
                            else:
                                s_t = spool.tile([128, 128], F32, tag="s")
                                nc.vector.tensor_scalar(
                                    out=s_t[:], in0=iota[:],
                                    scalar1=dstf[:, t:t + 1],
                                    scalar2=s2f[:, t:t + 1],
                                    op0=mybir.AluOpType.is_equal,
                                    op1=mybir.AluOpType.mult)
                                s_ap = s_t
                            if not NOMM:
                                rhs_ap = s_ap if isinstance(s_ap, bass.AP) else s_ap[:]
                                nc.tensor.matmul(
                                    out=psT[:], lhsT=gb[:, j * 128:(j + 1) * 128],
                                    rhs=rhs_ap,
                                    start=(t == tb0), stop=(t == tb1 - 1))
                        bs = slice(b * 128, (b + 1) * 128)
                        if NOMM:
                            continue
                        if q == first_q[b]:
                            nc.vector.tensor_copy(out=accT[:, bs], in_=psT[:])
                        else:
                            nc.vector.tensor_tensor(
                                out=accT[:, bs], in0=accT[:, bs], in1=psT[:],
                                op=mybir.AluOpType.add)

                ls = slice((l % L) * H, (l % L + 1) * H)
                for b in range(NBLK):
                    bs = slice(b * 128, (b + 1) * 128)
                    ps2 = p2.tile([128, 128], F32, tag="ps2")
                    nc.tensor.matmul(out=ps2[:], lhsT=accT[:, bs], rhs=wp[:, ls],
                                     start=True, stop=False)
                    nc.tensor.matmul(out=ps2[:], lhsT=h0T[:, bs], rhs=w01[:, ls],
                                     start=False, stop=True)
                    if l < NL - 1:
                        gn = epool.tile([128, H], F32, tag="gnew")
                        nc.scalar.activation(
                            out=gn[:], in_=ps2[:],
                            func=mybir.ActivationFunctionType.Relu,
                            scale=dvb[:, b:b + 1])
                        nc.sync.dma_start(out=ag_in[bs, :], in_=gn[:])
                    else:
                        ob = epool.tile([128, H], F32, tag="gnew")
                        nc.scalar.copy(out=ob[:], in_=ps2[:])
                        nc.sync.dma_start(out=out_t[bs, :], in_=ob[:])
                if l < NL - 1 and not NOCC:
                    nc.gpsimd.collective_compute(
                        "AllGather", mybir.AluOpType.bypass,
                        replica_groups=groups,
                        ins=[ag_in[:]], outs=[g_full[(l + 1) % 2][:]])

    nc.compile()
    return nc


def kernel(x, fc_w, fc_b, Ws, edge_index):
    global LAST_RESULT
    x = np.asarray(x, dtype=np.float32)
    fc_w = np.asarray(fc_w, dtype=np.float32)
    fc_b = np.asarray(fc_b, dtype=np.float32)
    Ws = np.asarray(Ws, dtype=np.float32)

    per_core, T, nt, off = _preprocess(edge_index)

    eye = np.eye(H, dtype=np.float32)
    wp = np.empty((L * H, H), dtype=np.float32)
    for l in range(1, L + 1):
        theta = math.log(LAMDA / l + 1.0)
        wp[(l - 1) * H:l * H] = theta * Ws[l - 1] + (1.0 - theta) * eye
    w01 = (ALPHA * wp).astype(np.float32)

    iota = np.broadcast_to(np.arange(128, dtype=np.float32), (128, 128)).copy()
    ident = np.eye(128, dtype=np.float32)
    fcb2 = fc_b.reshape(H, 1).copy()

    nc = _build(T, nt, off)

    in_maps = []
    for k in range(NCORES):
        xT = np.zeros((FIN, SPAD), dtype=np.float32)
        xT[:, :SHARD] = x[k * SHARD:(k + 1) * SHARD].T
        pc = per_core[k]
        in_maps.append(dict(
            xT=xT, fcw=fc_w, fcb=fcb2, wp=wp, w01=w01,
            colx=pc["colx16"], dstf=pc["dstf"], s2f=pc["s2f"], dvb=pc["dvb"],
            iotaf=iota, ident=ident,
        ))

    res = run_bass_kernel_spmd(nc, in_maps, core_ids=list(range(NCORES)))
    LAST_RESULT = res
    global LAST_NC, LAST_IN_MAPS
    LAST_NC, LAST_IN_MAPS = nc, in_maps

    out = np.concatenate(
        [res.results[k]["out"][:SHARD] for k in range(NCORES)], axis=0)
    return out


def time_exec(iters=3):
    """Best-effort device-time measurement: stage inputs on device, run the
    compiled NEFF repeatedly, report best wall-clock (includes a fixed
    multi-core dispatch overhead of the axon path, ~50-100ms)."""
    import time as _time
    import jax
    from jax.sharding import Mesh, PartitionSpec, NamedSharding
    from jax.experimental.shard_map import shard_map
    from concourse.bass2jax import (_bass_exec_p, install_neuronx_cc_hook,
                                    partition_id_tensor)
    nc, in_maps = LAST_NC, LAST_IN_MAPS
    if nc is None:
        return None
    install_neuronx_cc_hook()
    pname = nc.partition_id_tensor.name if nc.partition_id_tensor else None
    in_names, out_names, out_avals, zeros = [], [], [], []
    for alloc in nc.m.functions[0].allocations:
        if not isinstance(alloc, mybir.MemoryLocationSet):
            continue
        nm = alloc.memorylocations[0].name
        if alloc.kind == "ExternalInput":
            if nm != pname:
                in_names.append(nm)
        elif alloc.kind == "ExternalOutput":
            out_names.append(nm)
            out_avals.append(__import__("jax").core.ShapedArray(
                tuple(alloc.tensor_shape), mybir.dt.np(alloc.dtype)))
            zeros.append(np.zeros(tuple(alloc.tensor_shape),
                                  mybir.dt.np(alloc.dtype)))
    all_in = list(in_names) + list(out_names)
    if pname is not None:
        all_in.append(pname)

    def _body(*args):
        ops = list(args)
        if pname is not None:
            ops.append(partition_id_tensor())
        return tuple(_bass_exec_p.bind(
            *ops, out_avals=tuple(out_avals), in_names=tuple(all_in),
            out_names=tuple(out_names), lowering_input_output_aliases=(),
            sim_require_finite=True, sim_require_nnan=True, nc=nc))

    devs = jax.devices()[:NCORES]
    mesh = Mesh(np.asarray(devs), ("core",))
    n = len(in_names) + len(out_names)
    jf = jax.jit(shard_map(_body, mesh=mesh,
                           in_specs=(PartitionSpec("core"),) * n,
                           out_specs=(PartitionSpec("core"),) * len(out_names),
                           check_rep=False), keep_unused=True)
    sh = NamedSharding(mesh, PartitionSpec("core"))
    din = [jax.device_put(np.concatenate(
        [np.asarray(m[nm]) for m in in_maps], axis=0), sh) for nm in in_names]
    dz = [jax.device_put(np.zeros((NCORES * z.shape[0], *z.shape[1:]), z.dtype), sh)
          for z in zeros]
    jax.block_until_ready(din + dz)
    out = jf(*din, *dz); jax.block_until_ready(out)
    best = None
    for _ in range(iters):
        t0 = _time.time()
        out = jf(*din, *dz); jax.block_until_ready(out)
        dt = _time.time() - t0
        best = dt if best is None or dt < best else best
    return int(best * 1e9)


# revision 17
# speedup vs baseline: 1.5554x; 1.0076x over previous
"""GCNII message-passing kernel for 8 Trainium2 NeuronCores.

Strategy (dest-sharded SpMM, indicator-matmul segment sum, dma_gather):
  - Nodes sharded across 8 cores (12500 each, padded to 12544 = 98*128).
  - Edges partitioned by destination core. Within a core, edges are
    grouped by (source-quarter q, 128-dest block b) and padded into
    128-edge tiles; tiles are laid out quarter-major so each dma_gather
    batch reads one 25088-row sub-table with int16 indices.
  - Self-loop term folded in as explicit self-edges.
  - g = dinv * h is the gathered quantity, so the edge weight
    norm_e = dinv[dst]*dinv[src] factors as (0.9*dinv[dst]) -> folded
    into the indicator matrix S[e,d] = (iota==dst_local)*0.9*dinv[dst]
    (one DVE tensor_scalar per tile).
  - Per tile one PE matmul accumulates sumT[h,d] += G[e,h]^T S[e,d] in
    PSUM; per (q,b) the PSUM partial is added into an SBUF accumulator.
  - Block epilogue: out[d,:] = accT^T... = lhsT(accT)@W' + lhsT(h0T)@(0.1W')
    via two PSUM-accumulated matmuls (W' = theta*W + (1-theta)*I), then
    g_next = dinv * relu(out) on the scalar engine.
  - g is all-gathered across cores between layers (ping-pong buffers).
"""

import math

import numpy as np

import concourse.bass as bass
import concourse.bacc as bacc
import concourse.mybir as mybir
import concourse.tile as tile
from concourse import library_config
from concourse.bass_utils import run_bass_kernel_spmd

N = 100000
FIN = 256
H = 128
L = 4
NCORES = 8
SHARD = N // NCORES            # 12500
NBLK = (SHARD + 127) // 128    # 98
SPAD = NBLK * 128              # 12544
NGF = NCORES * SPAD            # 100352 rows in gathered g table
NQ = 4
QROW = NGF // NQ               # 25088 rows per int16-addressable sub-table
LAMDA = 0.5
ALPHA = 0.1
import os
KB = int(os.environ.get("GCN_KB", "16"))  # edge tiles per batched dma_gather
NL = int(os.environ.get("GCN_L", "4"))    # layers to actually run (bisect)
NOCC = os.environ.get("GCN_NOCC", "0") == "1"     # skip collectives (bisect)
NOGATHER = os.environ.get("GCN_NOGATHER", "0") == "1"  # plain DMA instead of gather
NOS = os.environ.get("GCN_NOS", "0") == "1"       # skip S-build (bisect)
NOMM = os.environ.get("GCN_NOMM", "0") == "1"     # skip edge matmuls (bisect)
SP = os.environ.get("GCN_SP", "0") == "1"         # dma_gather single_packet
WS = int(os.environ.get("GCN_WS", "8"))           # wide S-build chunk (0=per-tile)
NQG = int(os.environ.get("GCN_NQ", "1"))          # swdge queues for gathers

F32 = mybir.dt.float32
I16 = mybir.dt.int16

LAST_RESULT = None
LAST_NC = None
LAST_IN_MAPS = None


CCH = 2                        # AllGather chunks (overlap with epilogue)
CROWS = SPAD // CCH            # 6272 rows = 49 blocks per chunk


def _pg(v):
    """global node id -> row id in the chunk-major all-gathered g table:
    chunk c holds [8 cores x 6272 rows] contiguously."""
    k = v // SHARD
    r = v % SHARD
    c = r // CROWS
    return c * (NCORES * CROWS) + k * CROWS + (r % CROWS)


def _preprocess(edge_index):
    ei = np.asarray(edge_index).astype(np.int64)
    row, col = ei[0], ei[1]

    deg = (np.bincount(row, minlength=N) + 1.0).astype(np.float32)
    dinv = (1.0 / np.sqrt(deg.astype(np.float64))).astype(np.float32)

    core_of = row // SHARD
    selfd = np.arange(SHARD, dtype=np.int64)

    edata = []
    counts = np.zeros((NCORES, NQ * NBLK), dtype=np.int64)
    for k in range(NCORES):
        m = core_of == k
        dl = row[m] - k * SHARD
        src = col[m]
        a_dl = np.concatenate([dl, selfd])
        a_srcpg = _pg(np.concatenate([src, k * SHARD + selfd]))
        a_key = (a_srcpg // QROW) * NBLK + (a_dl >> 7)
        counts[k] = np.bincount(a_key, minlength=NQ * NBLK)
        edata.append((a_dl, a_srcpg, a_key))

    nt = (counts.max(axis=0) + 127) // 128          # [NQ*NBLK] tiles per group
    off = np.concatenate([[0], np.cumsum(nt)])      # tile offset per group
    T = int(off[-1])

    per_core = []
    for k in range(NCORES):
        a_dl, a_srcpg, a_key = edata[k]
        order = np.argsort(a_key, kind="stable")
        a_dl, a_srcpg, a_key = a_dl[order], a_srcpg[order], a_key[order]
        starts = np.concatenate([[0], np.cumsum(counts[k])])
        rank = np.arange(len(a_key)) - starts[a_key]
        slot = off[a_key] * 128 + rank

        colx = np.zeros(T * 128, dtype=np.int16)
        dstf = np.full(T * 128, -1.0, dtype=np.float32)
        s2f = np.zeros(T * 128, dtype=np.float32)
        colx[slot] = (a_srcpg % QROW).astype(np.int16)
        dstf[slot] = (a_dl & 127).astype(np.float32)
        s2f[slot] = (1.0 - ALPHA) * dinv[k * SHARD + a_dl]

        # idx wrap: logical i at [i%16, i//16], replicated to 128 partitions
        colx16 = np.tile(colx.reshape(T * 8, 16).T, (8, 1))   # [128, 8T]
        dstf = np.ascontiguousarray(dstf.reshape(T, 128).T)   # [128, T]
        s2f = np.ascontiguousarray(s2f.reshape(T, 128).T)

        dv = np.zeros(SPAD, dtype=np.float32)
        dv[:SHARD] = dinv[k * SHARD:(k + 1) * SHARD]
        dvb = np.ascontiguousarray(dv.reshape(NBLK, 128).T)   # [128, NBLK]

        per_core.append(dict(colx16=np.ascontiguousarray(colx16),
                             dstf=dstf, s2f=s2f, dvb=dvb))

    return per_core, T, nt, off


def _build(T, nt, off):
    nc = bacc.Bacc("TRN2", target_bir_lowering=False, debug=False,
                   num_devices=NCORES, num_swdge_queues=NQG)

    xT_t = nc.dram_tensor("xT", [FIN, SPAD], F32, kind="ExternalInput")
    fcw_t = nc.dram_tensor("fcw", [FIN, H], F32, kind="ExternalInput")
    fcb_t = nc.dram_tensor("fcb", [H, 1], F32, kind="ExternalInput")
    wp_t = nc.dram_tensor("wp", [L * H, H], F32, kind="ExternalInput")
    w01_t = nc.dram_tensor("w01", [L * H, H], F32, kind="ExternalInput")
    colx_t = nc.dram_tensor("colx", [128, 8 * T], I16, kind="ExternalInput")
    dstf_t = nc.dram_tensor("dstf", [128, T], F32, kind="ExternalInput")
    s2f_t = nc.dram_tensor("s2f", [128, T], F32, kind="ExternalInput")
    dvb_t = nc.dram_tensor("dvb", [128, NBLK], F32, kind="ExternalInput")
    iota_t = nc.dram_tensor("iotaf", [128, 128], F32, kind="ExternalInput")
    ident_t = nc.dram_tensor("ident", [128, 128], F32, kind="ExternalInput")
    out_t = nc.dram_tensor("out", [SPAD, H], F32, kind="ExternalOutput")

    ag_in = [nc.dram_tensor(f"ag_in{c}", [CROWS, H], F32)
             for c in range(CCH)]
    g_full = [nc.dram_tensor(f"g_full{i}", [NGF, H], F32, addr_space="Shared")
              for i in range(2)]

    groups = [list(range(NCORES))]

    # first quarter with any tiles, per block (same across cores)
    first_q = [min(q for q in range(NQ) if nt[q * NBLK + b] > 0)
               for b in range(NBLK)]

    with tile.TileContext(nc) as tc:
        with (
            tc.tile_pool(name="const", bufs=1) as cpool,
            tc.tile_pool(name="meta", bufs=1) as mpool,
            tc.tile_pool(name="h0", bufs=1) as hpool,
            tc.tile_pool(name="acc", bufs=1) as apool,
            tc.tile_pool(name="xc", bufs=3) as xpool,
            tc.tile_pool(name="ix", bufs=4) as ipool,
            tc.tile_pool(name="gb", bufs=3) as gpool,
            tc.tile_pool(name="s", bufs=6) as spool,
            tc.tile_pool(name="eout", bufs=6) as epool,
            tc.tile_pool(name="pT", bufs=6, space="PSUM") as pT,
            tc.tile_pool(name="p2", bufs=2, space="PSUM") as p2,
        ):
            nc.gpsimd.load_library(library_config.mlp)

            iota = cpool.tile([128, 128], F32)
            nc.sync.dma_start(out=iota[:], in_=iota_t[:])
            ident = cpool.tile([128, 128], F32)
            nc.sync.dma_start(out=ident[:], in_=ident_t[:])
            fcw0 = cpool.tile([128, H], F32)
            nc.sync.dma_start(out=fcw0[:], in_=fcw_t[0:128, :])
            fcw1 = cpool.tile([128, H], F32)
            nc.sync.dma_start(out=fcw1[:], in_=fcw_t[128:256, :])
            fcb = cpool.tile([128, 1], F32)
            nc.sync.dma_start(out=fcb[:], in_=fcb_t[:])
            dvb = cpool.tile([128, NBLK], F32)
            nc.sync.dma_start(out=dvb[:], in_=dvb_t[:])
            wp = cpool.tile([128, L * H], F32)
            w01 = cpool.tile([128, L * H], F32)
            for l in range(L):
                nc.sync.dma_start(out=wp[:, l * H:(l + 1) * H],
                                  in_=wp_t[l * H:(l + 1) * H, :])
                nc.sync.dma_start(out=w01[:, l * H:(l + 1) * H],
                                  in_=w01_t[l * H:(l + 1) * H, :])
            dstf = mpool.tile([128, T], F32)
            nc.sync.dma_start(out=dstf[:], in_=dstf_t[:])
            s2f = mpool.tile([128, T], F32)
            nc.sync.dma_start(out=s2f[:], in_=s2f_t[:])

            h0T = hpool.tile([128, SPAD], F32)
            accT = apool.tile([128, SPAD], F32)

            # ---- phase A: h0T = fcw^T @ xT + b ; g0 = dinv*h0 ---------
            for b in range(NBLK):
                cs = slice(b * 128, (b + 1) * 128)
                xc0 = xpool.tile([128, 128], F32, tag="xc0")
                nc.sync.dma_start(out=xc0[:], in_=xT_t[0:128, cs])
                xc1 = xpool.tile([128, 128], F32, tag="xc1")
                nc.sync.dma_start(out=xc1[:], in_=xT_t[128:256, cs])
                ph = pT.tile([128, 128], F32, tag="psT")
                nc.tensor.matmul(out=ph[:], lhsT=fcw0[:], rhs=xc0[:],
                                 start=True, stop=False)
                nc.tensor.matmul(out=ph[:], lhsT=fcw1[:], rhs=xc1[:],
                                 start=False, stop=True)
                nc.vector.tensor_scalar(out=h0T[:, cs], in0=ph[:],
                                        scalar1=fcb[:, :1], scalar2=None,
                                        op0=mybir.AluOpType.add)
                pg0 = pT.tile([128, 128], F32, tag="psT")
                nc.tensor.transpose(out=pg0[:], in_=h0T[:, cs], identity=ident[:])
                g0 = epool.tile([128, H], F32, tag="gnew")
                nc.scalar.activation(out=g0[:], in_=pg0[:],
                                     func=mybir.ActivationFunctionType.Copy,
                                     scale=dvb[:, b:b + 1])
                c = b // 49
                rs0 = (b - c * 49) * 128
                nc.sync.dma_start(out=ag_in[c][rs0:rs0 + 128, :], in_=g0[:])

            if not NOCC:
                for c in range(CCH):
                    nc.gpsimd.collective_compute(
                        "AllGather", mybir.AluOpType.bypass,
                        replica_groups=groups, ins=[ag_in[c][:]],
                        outs=[g_full[0][c * NCORES * CROWS:
                                        (c + 1) * NCORES * CROWS, :]])

            # ---- phase B: 4 GCNII layers ------------------------------
            for l in range(NL):
                gsrc = g_full[l % 2]
                gbufs = {}
                gq_i = 0
                for q in range(NQ):
                    qt0, qt1 = int(off[q * NBLK]), int(off[(q + 1) * NBLK])
                    gq = gsrc[q * QROW:(q + 1) * QROW, :]
                    for t0 in range(qt0, qt1, KB):
                        t1 = min(qt1, t0 + KB)
                        nb = t1 - t0
                        ix = ipool.tile([128, KB * 8], I16, tag="ix")
                        nc.sync.dma_start(out=ix[:, :nb * 8],
                                          in_=colx_t[:, t0 * 8:t1 * 8])
                        gb = gpool.tile([128, KB * 128], F32, tag="gb")
                        if NOGATHER:
                            nc.sync.dma_start(
                                out=gb[:, :nb * 128].rearrange("p (a b) -> p a b", b=128),
                                in_=gq[0:nb * 128, :].rearrange("(a p) b -> p a b", p=128))
                        else:
                            nc.gpsimd.dma_gather(
                                gb[:, :nb * 128].rearrange("p (a b) -> p a b", b=128),
                                gq, ix[:, :nb * 8], nb * 128, nb * 128, 128,
                                single_packet=SP, queue_num=gq_i % NQG)
                            gq_i += 1
                        gbufs[t0] = gb

                    for b in range(NBLK):
                        g_i = q * NBLK + b
                        tb0, tb1 = int(off[g_i]), int(off[g_i + 1])
                        if tb1 == tb0:
                            continue
                        psT = pT.tile([128, 128], F32, tag="psT")
                        swide = {}
                        if WS and not NOS:
                            for w0 in range(tb0, tb1, WS):
                                w1 = min(tb1, w0 + WS)
                                nw = w1 - w0
                                sw = spool.tile([128, WS * 128], F32, tag="sw")
                                sw3 = sw[:, :nw * 128].rearrange(
                                    "p (a f) -> p a f", f=128)
                                nc.vector.tensor_tensor(
                                    out=sw3,
                                    in0=iota[:].rearrange("p (a f) -> p a f", a=1)
                                        .to_broadcast([128, nw, 128]),
                                    in1=dstf[:, w0:w1].rearrange(
                                        "p (a f) -> p a f", f=1)
                                        .to_broadcast([128, nw, 128]),
                                    op=mybir.AluOpType.is_equal)
                                nc.vector.tensor_tensor(
                                    out=sw3, in0=sw3,
                                    in1=s2f[:, w0:w1].rearrange(
                                        "p (a f) -> p a f", f=1)
                                        .to_broadcast([128, nw, 128]),
                                    op=mybir.AluOpType.mult)
                                swide[w0] = sw
                        for t in range(tb0, tb1):
                            bstart = qt0 + ((t - qt0) // KB) * KB
                            gb = gbufs[bstart]
                            j = t - bstart
                            if WS and not NOS:
                                w0 = tb0 + ((t - tb0) // WS) * WS
                                sw = swide[w0]
                                s_ap = sw[:, (t - w0) * 128:(t - w0 + 1) * 128]
                            elif NOS:
                                s_t = spool.tile([128, 128], F32, tag="s")
                                if t == tb0:
                                    nc.vector.tensor_copy(out=s_t[:], in_=iota[:])
                                s_ap = iota
                            else:
                                s_t = spool.tile([128, 128], F32, tag="s")
                                nc.vector.tensor_scalar(
                                    out=s_t[:], in0=iota[:],
                                    scalar1=dstf[:, t:t + 1],
                                    scalar2=s2f[:, t:t + 1],
                                    op0=mybir.AluOpType.is_equal,
                                    op1=mybir.AluOpType.mult)
                                s_ap = s_t
                            if not NOMM:
                                rhs_ap = s_ap if isinstance(s_ap, bass.AP) else s_ap[:]
                                nc.tensor.matmul(
                                    out=psT[:], lhsT=gb[:, j * 128:(j + 1) * 128],
                                    rhs=rhs_ap,
                                    start=(t == tb0), stop=(t == tb1 - 1))
                        bs = slice(b * 128, (b + 1) * 128)
                        if NOMM:
                            continue
                        if q == first_q[b]:
                            nc.any.tensor_copy(out=accT[:, bs], in_=psT[:])
                        else:
                            nc.any.tensor_tensor(
                                out=accT[:, bs], in0=accT[:, bs], in1=psT[:],
                                op=mybir.AluOpType.add)

                ls = slice((l % L) * H, (l % L + 1) * H)
                for b in range(NBLK):
                    bs = slice(b * 128, (b + 1) * 128)
                    ps2 = p2.tile([128, 128], F32, tag="ps2")
                    nc.tensor.matmul(out=ps2[:], lhsT=accT[:, bs], rhs=wp[:, ls],
                                     start=True, stop=False)
                    nc.tensor.matmul(out=ps2[:], lhsT=h0T[:, bs], rhs=w01[:, ls],
                                     start=False, stop=True)
                    if l < NL - 1:
                        gn = epool.tile([128, H], F32, tag="gnew")
                        nc.scalar.activation(
                            out=gn[:], in_=ps2[:],
                            func=mybir.ActivationFunctionType.Relu,
                            scale=dvb[:, b:b + 1])
                        c = b // 49
                        rs0 = (b - c * 49) * 128
                        nc.sync.dma_start(out=ag_in[c][rs0:rs0 + 128, :],
                                          in_=gn[:])
                        if b % 49 == 48 and not NOCC:
                            nc.gpsimd.collective_compute(
                                "AllGather", mybir.AluOpType.bypass,
                                replica_groups=groups, ins=[ag_in[c][:]],
                                outs=[g_full[(l + 1) % 2]
                                      [c * NCORES * CROWS:
                                       (c + 1) * NCORES * CROWS, :]])
                    else:
                        ob = epool.tile([128, H], F32, tag="gnew")
                        nc.scalar.copy(out=ob[:], in_=ps2[:])
                        nc.sync.dma_start(out=out_t[bs, :], in_=ob[:])


    nc.compile()
    return nc


def kernel(x, fc_w, fc_b, Ws, edge_index):
    global LAST_RESULT
    x = np.asarray(x, dtype=np.float32)
    fc_w = np.asarray(fc_w, dtype=np.float32)
    fc_b = np.asarray(fc_b, dtype=np.float32)
    Ws = np.asarray(Ws, dtype=np.float32)

    per_core, T, nt, off = _preprocess(edge_index)

    eye = np.eye(H, dtype=np.float32)
    wp = np.empty((L * H, H), dtype=np.float32)
    for l in range(1, L + 1):
        theta = math.log(LAMDA / l + 1.0)
        wp[(l - 1) * H:l * H] = theta * Ws[l - 1] + (1.0 - theta) * eye
    w01 = (ALPHA * wp).astype(np.float32)

    iota = np.broadcast_to(np.arange(128, dtype=np.float32), (128, 128)).copy()
    ident = np.eye(128, dtype=np.float32)
    fcb2 = fc_b.reshape(H, 1).copy()

    nc = _build(T, nt, off)

    in_maps = []
    for k in range(NCORES):
        xT = np.zeros((FIN, SPAD), dtype=np.float32)
        xT[:, :SHARD] = x[k * SHARD:(k + 1) * SHARD].T
        pc = per_core[k]
        in_maps.append(dict(
            xT=xT, fcw=fc_w, fcb=fcb2, wp=wp, w01=w01,
            colx=pc["colx16"], dstf=pc["dstf"], s2f=pc["s2f"], dvb=pc["dvb"],
            iotaf=iota, ident=ident,
        ))

    res = run_bass_kernel_spmd(nc, in_maps, core_ids=list(range(NCORES)))
    LAST_RESULT = res
    global LAST_NC, LAST_IN_MAPS
    LAST_NC, LAST_IN_MAPS = nc, in_maps

    out = np.concatenate(
        [res.results[k]["out"][:SHARD] for k in range(NCORES)], axis=0)
    return out


def time_exec(iters=3):
    """Best-effort device-time measurement: stage inputs on device, run the
    compiled NEFF repeatedly, report best wall-clock (includes a fixed
    multi-core dispatch overhead of the axon path, ~50-100ms)."""
    import time as _time
    import jax
    from jax.sharding import Mesh, PartitionSpec, NamedSharding
    from jax.experimental.shard_map import shard_map
    from concourse.bass2jax import (_bass_exec_p, install_neuronx_cc_hook,
                                    partition_id_tensor)
    nc, in_maps = LAST_NC, LAST_IN_MAPS
    if nc is None:
        return None
    install_neuronx_cc_hook()
    pname = nc.partition_id_tensor.name if nc.partition_id_tensor else None
    in_names, out_names, out_avals, zeros = [], [], [], []
    for alloc in nc.m.functions[0].allocations:
        if not isinstance(alloc, mybir.MemoryLocationSet):
            continue
        nm = alloc.memorylocations[0].name
        if alloc.kind == "ExternalInput":
            if nm != pname:
                in_names.append(nm)
        elif alloc.kind == "ExternalOutput":
            out_names.append(nm)
            out_avals.append(__import__("jax").core.ShapedArray(
                tuple(alloc.tensor_shape), mybir.dt.np(alloc.dtype)))
            zeros.append(np.zeros(tuple(alloc.tensor_shape),
                                  mybir.dt.np(alloc.dtype)))
    all_in = list(in_names) + list(out_names)
    if pname is not None:
        all_in.append(pname)

    def _body(*args):
        ops = list(args)
        if pname is not None:
            ops.append(partition_id_tensor())
        return tuple(_bass_exec_p.bind(
            *ops, out_avals=tuple(out_avals), in_names=tuple(all_in),
            out_names=tuple(out_names), lowering_input_output_aliases=(),
            sim_require_finite=True, sim_require_nnan=True, nc=nc))

    devs = jax.devices()[:NCORES]
    mesh = Mesh(np.asarray(devs), ("core",))
    n = len(in_names) + len(out_names)
    jf = jax.jit(shard_map(_body, mesh=mesh,
                           in_specs=(PartitionSpec("core"),) * n,
                           out_specs=(PartitionSpec("core"),) * len(out_names),
                           check_rep=False), keep_unused=True)
    sh = NamedSharding(mesh, PartitionSpec("core"))
    din = [jax.device_put(np.concatenate(
        [np.asarray(m[nm]) for m in in_maps], axis=0), sh) for nm in in_names]
    dz = [jax.device_put(np.zeros((NCORES * z.shape[0], *z.shape[1:]), z.dtype), sh)
          for z in zeros]
    jax.block_until_ready(din + dz)
    out = jf(*din, *dz); jax.block_until_ready(out)
    best = None
    for _ in range(iters):
        t0 = _time.time()
        out = jf(*din, *dz); jax.block_until_ready(out)
        dt = _time.time() - t0
        best = dt if best is None or dt < best else best
    return int(best * 1e9)
